# revision 1
# baseline (speedup 1.0000x reference)
"""Trainium2 Bass kernel for nn_ModelPaperBaseline_bin (binarized CNN).

Contract: kernel(**inputs) takes FULL unsharded inputs (batch 65536) and
returns the FULL (65536, 1) float32 output. Internally shards the batch
across 8 NeuronCores (pure data parallel), runs one SPMD Bass program.

Network (per sample):
  x (4,16) -> conv0 1x1 (bin W) -> BN -> sign -> s0 (32,16)   [shortcut]
  4x { conv3x1 pad1 (bin W) -> BN -> sign -> + s0 }
  flatten (512) -> fc1 (bin W) -> BN -> sign -> (64)
  fc2 (bin W) -> BN -> sign -> (64) -> fc3 (bin W) -> sigmoid -> (1)

Key tricks vs the v0 kernel (kernel_v0.py):
- conv0 via a single K=128 fp16 matmul per quad: x is split into two fp16
  components (h0 + h1 covers ~22 mantissa bits; residual ~2^-24 abs, which
  flips ~1 near-threshold sample in 65536 - same class as fp32 sum-order).
- Everything after conv0 is exact small integers, so all other matmuls run
  in fp8e4 with DoubleRow perf mode (0.5 PE cycles per output column,
  256-row virtual contraction via a [K,2,N] access pattern).
- The residual add never happens on a vector engine: y_l = sign_l + s0 and
  conv is linear, so conv(y) = conv(sign_l) + conv(s0) rides the matmul as
  the second DoubleRow slot with the same (per-slot-scaled) weights. The
  sign tiles s0, st1..st4 live as slots of one [128, 5, cf] mega-tile so
  any pair is a strided 3D access pattern.
- Threshold ops are one instruction on any engine: ScalarE Sign -> +-1
  (weight scale 1), DVE/GpSimd (z>=thr)-0.5 -> +-0.5 (weight scale 2 baked
  into the consuming matmul's fp8 weights - numerically identical).
- Cross-quad conv taps are 32-row DoubleRow matmuls at tile_position
  (96,0)/(0,96) accumulating into the same PSUM banks.
- fc stage packs two 512-sample halves into the 128-partition dim
  (block-diagonal fc2/fc3 weights), halving fc matmul columns and
  activation sizes.

On-chip layout: partition p = 32*j + c (j = position-in-quad, c = channel),
column n = g*S + b (g = quad index 0..3, b = sample-in-chunk). W=16 = 4
quads of 4 positions.
"""

import os
import sys

sys.path.insert(0, "/opt/trn_rl_repo")

import numpy as np
import ml_dtypes

BF16 = ml_dtypes.bfloat16
FP16 = np.float16
FP8 = ml_dtypes.float8_e4m3
EPS = 0.01

B_TOTAL = 65536
N_CORES = 8
B_CORE = B_TOTAL // N_CORES  # 8192
S = int(os.environ.get("KS", 256))  # samples per chunk
CF = 4 * S       # columns per activation tile (4 quads * S)
W = 16
C = 32
H = 64

# engine per threshold op: S=ScalarE(Sign,+-1) D=VectorE P=GpSimd (+-0.5,
# weight scale 2 folded into consuming matmuls). conv-stage entries are two
# chars (cf halves A/B); a tile's form must be uniform, so valid pairs are
# SS or any mix of D/P.
# Only ScalarE (S) and VectorE (D) can read PSUM on trn2 hardware;
# GpSimd and DMA cannot touch PSUM at all. One char per stage = one
# unsplit threshold op. st4 may be two chars (per-half engines): its
# halves feed disjoint fc1 matmuls (quads 0-1 vs 2-3), so the +-1/+-0.5
# forms can differ per half via per-half fc1 weight scales.
# s0 and st1 alternate engines per chunk pair (chunks 0-1 of a group use
# the configured engine, chunks 2-3 the other) so the group-boundary s0
# wave runs on both engines; the form difference is absorbed by
# chunk-parity weight-blob variants.
ENG_S0 = os.environ.get("E_S0", "S")
ENG_ST = os.environ.get("E_ST", "S,S,D,SD").split(",")  # layers 1..4
ENG_H1 = os.environ.get("E_H1", "D")
ENG_H2 = os.environ.get("E_H2", "D")
# ALT: which stages alternate engines per chunk pair ("0"=s0 .. "4"=st4)
ALT = os.environ.get("ALT", "0156")
for _e in [ENG_S0] + ENG_ST[0:3]:
    assert _e in ("S", "D", "SS", "DD"), _e
assert ENG_ST[3] in ("S", "D", "SS", "DD", "SD", "DS")


def _flip(e):
    return "".join({"S": "D", "D": "S"}.get(c, c) for c in e)


def _veng(stage, base, v):
    # engine of a stage for chunk-parity v
    if str(stage) in ALT:
        return base if v == 0 else _flip(base)
    return base


def _sgn(w):
    return np.where(w >= 0, 1.0, -1.0).astype(np.float32)


def _scale_of(eng):
    return 1.0 if eng == "S" else 2.0


# fp8 const blob layout (columns); V=2 chunk-parity variants
L1_OFF = 0                 # 2 x [128, 4*128] layer-1 [L, M, Z(zeros), R]
L2_OFF = 1024              # 3 x 2 x [128, 6*128] layers 2-4 (sc, st groups)
FC1_OFF = 1024 + 3 * 1536  # 4 quads x 2 halves x 2 var x [128, 2*128]
FC2_OFF = FC1_OFF + 4096   # 2 x [128, 3*128] block-diag fc2 as [W, 0, W]
FC3_OFF = FC2_OFF + 768    # 2 x [128, 3*128] fc3 as [w|0, 0, w|0] (cols 0:2)
FC3P_OFF = FC3_OFF + 768   # packed fc3: 2 par x 2 mm x [128, 2*128]
FC3T_OFF = FC3P_OFF + 1024  # tail-packed fc3 (w2=2s/2): 2 par x [128, 2*128]
CB8_COLS = FC3T_OFF + 512


def prepare_host_tensors(inp):
    """Fold BN into thresholds, binarize + arrange weights into lhsT blobs."""
    f32 = np.float32
    a0v = [_scale_of(_veng(0, ENG_S0, v)[0]) for v in range(2)]
    a1v = [_scale_of(_veng(1, ENG_ST[0], v)[0]) for v in range(2)]
    a23v = [[_scale_of(_veng(i + 1, ENG_ST[i], v)[0]) for v in range(2)]
            for i in (1, 2)]  # st2, st3 per parity
    al4v = [[_scale_of(_veng(4, ENG_ST[3], v)[0]),
             _scale_of(_veng(4, ENG_ST[3], v)[-1])] for v in range(2)]

    # thresholds: z >= thr  <=>  bn(z) >= 0 (BN scale > 0)
    s0s = inp["bn0_g"] / np.sqrt(inp["bn0_v"] + EPS)
    thr0 = inp["bn0_m"] - inp["bn0_b"] / s0s - inp["conv0_b"]      # [32]
    sls = inp["bns_g"] / np.sqrt(inp["bns_v"] + EPS)
    thrl = inp["bns_m"] - inp["bns_b"] / sls - inp["convs_b"]      # [4, 32]
    s5 = inp["bn5_g"] / np.sqrt(inp["bn5_v"] + EPS)
    thr5 = inp["bn5_m"] - inp["bn5_b"] / s5 - inp["fc1_b"]         # [64]
    s6 = inp["bn6_g"] / np.sqrt(inp["bn6_v"] + EPS)
    thr6 = inp["bn6_m"] - inp["bn6_b"] / s6 - inp["fc2_b"]         # [64]

    # conv0: z0[32jp+c', gS+b] = sum_{t,c,i} w0q[64t+16c+i, g, 32jp+c'] *
    #   x16[b, t, c, i];  w0 nonzero only at i == 4g+jp
    w0s = _sgn(inp["conv0_w"][:, :, 0])                            # [c'=32, c=4]
    w0q = np.zeros((128, 4, 128), f32)
    for g in range(4):
        for jp in range(4):
            i = 4 * g + jp
            for c in range(4):
                for t in range(2):
                    w0q[64 * t + 16 * c + i, g, 32 * jp:32 * jp + 32] = w0s[:, c]
    cw0 = np.ascontiguousarray(w0q.reshape(128, 512)).astype(FP16)

    # conv layers: main band M[32j+c, 32jp+c'] = W[c',c,j-jp+1], |j-jp|<=1
    # left tap  L[96+c, c']    = W[c',c,0]  (out jp=0 <- in j=3 of prev quad)
    # right tap R[c, 96+c']    = W[c',c,2]  (out jp=3 <- in j=0 of next quad)
    ws = _sgn(inp["convs_w"])                                      # [4,c',c,3]

    def lmr(l, scale):
        M = np.zeros((128, 128), f32)
        L = np.zeros((128, 128), f32)
        R = np.zeros((128, 128), f32)
        for j in range(4):
            for jp in range(4):
                if abs(j - jp) <= 1:
                    M[32 * j:32 * j + 32, 32 * jp:32 * jp + 32] = \
                        ws[l, :, :, j - jp + 1].T
        L[96:128, 0:32] = ws[l, :, :, 0].T
        R[0:32, 96:128] = ws[l, :, :, 2].T
        return L * scale, M * scale, R * scale

    cb8 = np.zeros((128, CB8_COLS), f32)
    # layer 1 (input s0, scale a0v[v]): [L, M, Z, R] per parity variant
    # (Z = zero block so the lone R contribution on [s,3s) can ride a
    # half-empty DoubleRow pair)
    for v in range(2):
        L, M, R = lmr(0, a0v[v])
        off = v * 512
        cb8[:, off:off + 128] = L
        cb8[:, off + 128:off + 256] = M
        cb8[:, off + 384:off + 512] = R
    # layers 2-4: [Lsc,Msc,Rsc, Lst,Mst,Rst] per variant; sc=s0 (a0v),
    # st=st_{l-1} (st1 alternates; st2/st3 fixed)
    for li in range(1, 4):
        for v in range(2):
            off = L2_OFF + (li - 1) * 1536 + v * 768
            stv = a1v[v] if li == 1 else a23v[li - 2][v]
            Lsc, Msc, Rsc = lmr(li, a0v[v])
            Lst, Mst, Rst = lmr(li, stv)
            for k, blk in enumerate([Lsc, Msc, Rsc, Lst, Mst, Rst]):
                cb8[:, off + 128 * k:off + 128 * (k + 1)] = blk

    # fc1: W1g[32j+c, h'] = sgn(fc1_w)[h', c*16+4g+j]; slots (sc*a0v,
    # st4*al4[half-of-g]) per (quad, dst-half, parity): [128, 2, 128] with
    # W in out-cols [64h, 64h+64) (full-dst DoubleRow)
    f1s = _sgn(inp["fc1_w"])                                       # [64, 512]
    for g in range(4):
        W1g = np.zeros((128, 64), f32)
        for j in range(4):
            for c in range(C):
                W1g[32 * j + c, :] = f1s[:, c * 16 + 4 * g + j]
        for half in range(2):
            for v in range(2):
                off = FC1_OFF + ((g * 2 + half) * 2 + v) * 256
                oc = 64 * half
                cb8[:, off + oc:off + oc + 64] = W1g * a0v[v]
                cb8[:, off + 128 + oc:off + 128 + oc + 64] = \
                    W1g * al4v[v][g // 2]

    # fc2 block-diag as [W, 0, W]: DoubleRow pairs (W,0) / (0,W) let each
    # col-half run at 0.5 cyc/col with the junk slot zero-weighted.
    # Two variants: h1's engine (hence form scale) alternates per group.
    f2s = _sgn(inp["fc2_w"])                                       # [h2', h1]
    for gp in range(2):
        b1 = _scale_of(_veng(5, ENG_H1, gp))
        base = FC2_OFF + gp * 384
        for w_off in (base, base + 256):
            cb8[0:64, w_off:w_off + 64] = f2s.T * b1
            cb8[64:128, w_off + 64:w_off + 128] = f2s.T * b1
    # fc3 as [w|0, 0, w|0] with 128-wide free dims (full-dst DoubleRow;
    # only out-cols 0:2 are nonzero); two variants for h2's alternation
    for gp in range(2):
        b2 = _scale_of(_veng(6, ENG_H2, gp))
        base = FC3_OFF + gp * 384
        f3s = _sgn(inp["fc3_w"][0]) * b2                           # [64]
        for w_off in (base, base + 256):
            cb8[0:64, w_off] = f3s
            cb8[64:128, w_off + 1] = f3s
    # packed fc3 for w2=512 batches: out[p, j] = v(sample p*128+j);
    # MM m slot s reads h2f col j+256m+128s -> halfA rows to p=2m+s,
    # halfB rows to p=4+2m+s. Quarters the sigmoid column count.
    for gp in range(2):
        b2 = _scale_of(_veng(6, ENG_H2, gp))
        f3s = _sgn(inp["fc3_w"][0]) * b2
        for m in range(2):
            base = FC3P_OFF + (gp * 2 + m) * 256
            for sl in range(2):
                cb8[0:64, base + sl * 128 + 2 * m + sl] = f3s
                cb8[64:128, base + sl * 128 + 4 + 2 * m + sl] = f3s
        # tail variant (w2=256): one MM, halfB lands at p=2+s
        base = FC3T_OFF + gp * 256
        for sl in range(2):
            cb8[0:64, base + sl * 128 + sl] = f3s
            cb8[64:128, base + sl * 128 + 2 + sl] = f3s

    # f32 consts [128, 16]:
    # 0: thr0 | 1: -thr0 | 2:6 thr_l | 6:10 -thr_l | 10: thr5(x2 tiled)
    # 11: -thr5 | 12: thr6 | 13: -thr6 | 14: b7 (rows 0:2)
    cf = np.zeros((128, 16), f32)
    cf[:, 0] = np.tile(thr0, 4)
    cf[:, 1] = -cf[:, 0]
    for li in range(4):
        cf[:, 2 + li] = np.tile(thrl[li], 4)
        cf[:, 6 + li] = -cf[:, 2 + li]
    cf[:, 10] = np.tile(thr5, 2)
    cf[:, 11] = -cf[:, 10]
    cf[:, 12] = np.tile(thr6, 2)
    cf[:, 13] = -cf[:, 12]
    cf[0:8, 14] = inp["fc3_b"][0]

    return {"cst_f32": cf, "cst_fp8": cb8.astype(FP8),
            "cst_fp16": cw0}


def split_fp16(x):
    """2-way fp16 split of fp32 x: x ~= h0 + h1 (residual ~2^-24 abs)."""
    x = np.asarray(x, np.float32)
    h0 = x.astype(FP16)
    h1 = (x - h0.astype(np.float32)).astype(FP16)
    return np.stack([h0, h1], 1)  # [b, 2, ...]


def build_nc(b_core=B_CORE, s=S):
    from concourse import bass, mybir, tile
    import bass_rust

    cf = 4 * s
    nchunk = b_core // s
    f32 = mybir.dt.float32
    fp16 = mybir.dt.float16
    fp8 = mybir.dt.float8e4
    AF = mybir.ActivationFunctionType
    ALU = mybir.AluOpType
    DR = mybir.MatmulPerfMode.DoubleRow

    nc = bass.Bass()
    x_d = nc.dram_tensor("x2", [b_core, 2, 4, 16], fp16, kind="ExternalInput")
    cf_d = nc.dram_tensor("cst_f32", [128, 16], f32, kind="ExternalInput")
    c8_d = nc.dram_tensor("cst_fp8", [128, CB8_COLS], fp8, kind="ExternalInput")
    cw_d = nc.dram_tensor("cst_fp16", [128, 512], fp16, kind="ExternalInput")
    out_d = nc.dram_tensor("out", [1, b_core], f32, kind="ExternalOutput")

    def dr_rhs(t2d, col0, delta, n):
        """[128, 2, n] DoubleRow rhs: slot i reads cols col0+i*delta."""
        ap = t2d[:, col0:col0 + n].copy()
        ap.ap = bass_rust.VecI64Pair(
            [list(ap.ap[0]), [delta, 2], [1, n]])
        return ap

    with tile.TileContext(nc) as tc:
        with (
            tc.tile_pool(name="const", bufs=1) as constp,
            tc.tile_pool(name="xin", bufs=int(os.environ.get("BUF_X", 4))) as xpool,
            tc.tile_pool(name="acts", bufs=int(os.environ.get("BUF_Y", 8))) as apool,
            tc.tile_pool(name="fcact", bufs=int(os.environ.get("BUF_F", 2))) as fpool,
            tc.tile_pool(name="outs", bufs=int(os.environ.get("BUF_O", 3))) as opool,
            tc.tile_pool(name="cpsum", bufs=int(os.environ.get("BUF_CP", 3)), space="PSUM") as cpsum,
            tc.tile_pool(name="fpsum", bufs=int(os.environ.get("BUF_FP", 2)), space="PSUM") as fpsum,
        ):
            # ---- constants to SBUF (three DMAs) ----
            cwt = constp.tile([128, 512], fp16, tag="cwt")
            nc.gpsimd.dma_start(cwt[:], cw_d[:])
            cft = constp.tile([128, 16], f32, tag="cft")
            nc.gpsimd.dma_start(cft[:], cf_d[:])
            c8t = constp.tile([128, CB8_COLS], fp8, tag="c8t")
            nc.gpsimd.dma_start(c8t[:], c8_d[:])

            thr0p, nthr0 = cft[:, 0:1], cft[:, 1:2]
            thrl = [cft[:, 2 + i:3 + i] for i in range(4)]
            nthrl = [cft[:, 6 + i:7 + i] for i in range(4)]
            thr5, nthr5 = cft[:, 10:11], cft[:, 11:12]
            thr6, nthr6 = cft[:, 12:13], cft[:, 13:14]
            b7 = cft[0:8, 14:15]

            lb1v = [c8t[:, L1_OFF + v * 512:L1_OFF + (v + 1) * 512]
                    .rearrange("p (k m) -> p k m", k=4) for v in range(2)]
            lblv = [[c8t[:, L2_OFF + i * 1536 + v * 768:
                         L2_OFF + i * 1536 + (v + 1) * 768]
                     .rearrange("p (k m) -> p k m", k=6) for v in range(2)]
                    for i in range(3)]
            wf1 = [[[c8t[:, FC1_OFF + ((g * 2 + h) * 2 + v) * 256:
                         FC1_OFF + ((g * 2 + h) * 2 + v + 1) * 256]
                     .rearrange("p (k m) -> p k m", k=2) for v in range(2)]
                    for h in range(2)] for g in range(4)]
            wf2v = [c8t[:, FC2_OFF + gp * 384:FC2_OFF + (gp + 1) * 384]
                    .rearrange("p (k m) -> p k m", k=3) for gp in range(2)]
            wf3v = [c8t[:, FC3_OFF + gp * 384:FC3_OFF + (gp + 1) * 384]
                    .rearrange("p (k m) -> p k m", k=3) for gp in range(2)]
            wf3p = [[c8t[:, FC3P_OFF + (gp * 2 + m) * 256:
                         FC3P_OFF + (gp * 2 + m + 1) * 256]
                     .rearrange("p (k m) -> p k m", k=2) for m in range(2)]
                    for gp in range(2)]
            wf3t = [c8t[:, FC3T_OFF + gp * 256:FC3T_OFF + (gp + 1) * 256]
                    .rearrange("p (k m) -> p k m", k=2) for gp in range(2)]

            xq = os.environ.get("XQ", "sp")

            def emit_xload(k):
                xt = xpool.tile([128, s], fp16, tag="xt")
                q = nc.gpsimd if xq == "pool" else nc.sync
                q.dma_start(
                    xt[:], x_d[k * s:(k + 1) * s].rearrange(
                        "b t c i -> (t c i) b"))
                return xt

            def emit_conv0(xt):
                z = cpsum.tile([128, cf], f32, tag="z", name="z")
                for g in range(4):
                    first = (g % 2 == 0)
                    nc.tensor.matmul(z[:, g * s:(g + 1) * s],
                                     cwt[:, g * 128:(g + 1) * 128], xt[:],
                                     start=first, stop=first,
                                     skip_group_check=not first)
                return z

            def emit_thresh(eng, dst, zp, thr_pos, thr_neg):
                if eng == "S":
                    nc.scalar.activation(dst, zp, AF.Sign, bias=thr_neg)
                else:
                    nc.vector.tensor_scalar(dst, zp, thr_pos, 0.5,
                                            ALU.is_ge, ALU.subtract)

            def emit_thresh_stage(cfg, mt, slot, z, thr_pos, thr_neg):
                if len(cfg) == 1:
                    emit_thresh(cfg, mt[:, slot, :], z[:, :],
                                thr_pos, thr_neg)
                else:
                    emit_thresh(cfg[0], mt[:, slot, 0:2 * s],
                                z[:, 0:2 * s], thr_pos, thr_neg)
                    emit_thresh(cfg[1], mt[:, slot, 2 * s:4 * s],
                                z[:, 2 * s:4 * s], thr_pos, thr_neg)

            def emit_conv1(mt, v):
                # layer 1: input s0 only (slot 0); intra-slot shifted pairs.
                # blob order [L@0, M@1, Z@2, R@3]; coverage:
                #   M on [0,4s), L on [s,4s), R on [0,3s) - each col once.
                z = cpsum.tile([128, cf], f32, tag="z", name="z")
                lb1 = lb1v[v]
                s0t = mt[:, 0, :]
                mm = nc.tensor.matmul
                # [0,s): (M,R) slots (n, n+s) - opens+closes bank0 group
                mm(z[:, 0:s], lb1[:, 1:4:2, :], dr_rhs(s0t, 0, s, s),
                   start=True, stop=True, perf_mode=DR)
                # [s,2s): (L,M) slots (n-s, n)
                mm(z[:, s:2 * s], lb1[:, 0:2, :], dr_rhs(s0t, 0, s, s),
                   start=False, stop=False, perf_mode=DR,
                   skip_group_check=True)
                # [s,2s): (Z,R) slots (junk, n+s)
                mm(z[:, s:2 * s], lb1[:, 2:4, :], dr_rhs(s0t, 0, 2 * s, s),
                   start=False, stop=False, perf_mode=DR,
                   skip_group_check=True)
                # [2s,3s): (L,M) - opens+closes bank1 group
                mm(z[:, 2 * s:3 * s], lb1[:, 0:2, :], dr_rhs(s0t, s, s, s),
                   start=True, stop=True, perf_mode=DR)
                # [2s,3s): (Z,R)
                mm(z[:, 2 * s:3 * s], lb1[:, 2:4, :],
                   dr_rhs(s0t, s, 2 * s, s),
                   start=False, stop=False, perf_mode=DR,
                   skip_group_check=True)
                # [3s,4s): (L,M)
                mm(z[:, 3 * s:4 * s], lb1[:, 0:2, :], dr_rhs(s0t, 2 * s, s, s),
                   start=False, stop=False, perf_mode=DR,
                   skip_group_check=True)
                return z

            def emit_convl(mt, li, v):
                # layers 2-4 (li=1..3): DR pairs (s0, st_li) = mt slots
                # (0, li); all weights from lbl[li-1]
                z = cpsum.tile([128, cf], f32, tag="z", name="z")
                lb = lblv[li - 1][v]
                mm = nc.tensor.matmul
                pair = mt[:, 0:li + 1:li, :] if li > 1 else mt[:, 0:2, :]
                # bank-A writers first so its threshold can start early:
                # M [0,2s); L out [s,2s) reads [0,s); R out [0,2s) reads
                # [s,3s). (Full-partition MMs: walrus requires DoubleRow dst
                # to be the full array; L/R are zero off the tap corner.)
                mm(z[:, 0:2 * s], lb[:, 1:5:3, :], pair[:, :, 0:2 * s],
                   start=True, stop=True, perf_mode=DR)
                mm(z[:, s:2 * s], lb[:, 0:4:3, :],
                   pair[:, :, 0:s],
                   start=False, stop=False, perf_mode=DR,
                   skip_group_check=True)
                mm(z[:, 0:2 * s], lb[:, 2:6:3, :],
                   pair[:, :, s:3 * s],
                   start=False, stop=False, perf_mode=DR,
                   skip_group_check=True)
                # bank-B: M [2s,4s); L out [2s,4s) reads [s,3s); R out
                # [2s,3s) reads [3s,4s)
                mm(z[:, 2 * s:4 * s], lb[:, 1:5:3, :], pair[:, :, 2 * s:4 * s],
                   start=True, stop=True, perf_mode=DR)
                mm(z[:, 2 * s:4 * s], lb[:, 0:4:3, :],
                   pair[:, :, s:3 * s],
                   start=False, stop=False, perf_mode=DR,
                   skip_group_check=True)
                mm(z[:, 2 * s:3 * s], lb[:, 2:6:3, :],
                   pair[:, :, 3 * s:4 * s],
                   start=False, stop=False, perf_mode=DR,
                   skip_group_check=True)
                return z

            fcb = int(os.environ.get("FCB", 4))

            def emit_fc(k0, mts, gp):
                for i in range(0, len(mts), fcb):
                    emit_fc_batch(k0 + i, mts[i:i + fcb], gp)

            def emit_fc_batch(k0, mts, gp):
                # 2n chunks -> two n*s-sample halves in the partition dim
                assert len(mts) % 2 == 0
                perhalf = len(mts) // 2
                w2 = perhalf * s
                hp = fpsum.tile([128, w2], f32, tag="hp")
                for ci, (mt, v) in enumerate(mts):
                    half, within = divmod(ci, perhalf)
                    oc = within * s
                    for g in range(4):
                        first = (g == 0 and within == 0 and half == 0)
                        nc.tensor.matmul(
                            hp[:, oc:oc + s],
                            wf1[g][half][v], mt[:, 0:5:4, g * s:(g + 1) * s],
                            start=first, stop=first,
                            perf_mode=DR,
                            skip_group_check=not first)
                h1f = fpool.tile([128, w2], fp8, tag="h1")
                eh1 = _veng(5, ENG_H1, gp)
                eh2 = _veng(6, ENG_H2, gp)
                wf2, wf3 = wf2v[gp], wf3v[gp]
                emit_thresh(eh1, h1f[:], hp[:], thr5, nthr5)
                h2p = fpsum.tile([128, w2], f32, tag="hp")
                hw2 = w2 // 2
                h1t = h1f[:, :]
                nc.tensor.matmul(h2p[:, 0:hw2], wf2[:, 0:2, :],
                                 dr_rhs(h1t, 0, hw2, hw2),
                                 start=True, stop=True, perf_mode=DR)
                nc.tensor.matmul(h2p[:, hw2:w2], wf2[:, 1:3, :],
                                 dr_rhs(h1t, 0, hw2, hw2),
                                 start=True, stop=True, perf_mode=DR,
                                 skip_group_check=True)
                h2f = fpool.tile([128, w2], fp8, tag="h2")
                emit_thresh(eh2, h2f[:], h2p[:], thr6, nthr6)
                h2t = h2f[:, :]
                if w2 == 2 * s:
                    # packed: out [8, 128], sigmoid over 128 cols only
                    op = fpsum.tile([128, 128], f32, tag="hp")
                    for m in range(2):
                        nc.tensor.matmul(op[:, 0:128], wf3p[gp][m],
                                         dr_rhs(h2t, 256 * m, 128, 128),
                                         start=(m == 0), stop=(m == 0),
                                         perf_mode=DR,
                                         skip_group_check=(m == 1))
                    ot = opool.tile([8, 128], f32, tag="ot")
                    nc.scalar.activation(ot[:], op[0:8, :], AF.Sigmoid,
                                         bias=b7[:, 0:1])
                    nc.gpsimd.dma_start(
                        out_d[0:1, k0 * s:k0 * s + 2 * w2].rearrange(
                            "o (p j) -> (o p) j", p=8), ot[:])
                else:
                    # tail batch (w2 == s): packed [4, 128]
                    op = fpsum.tile([128, 128], f32, tag="hp")
                    nc.tensor.matmul(op[:, 0:128], wf3t[gp],
                                     dr_rhs(h2t, 0, 128, 128),
                                     start=True, stop=True, perf_mode=DR)
                    ot = opool.tile([4, 128], f32, tag="ot")
                    nc.scalar.activation(ot[:], op[0:4, :], AF.Sigmoid,
                                         bias=b7[0:4, 0:1])
                    nc.gpsimd.dma_start(
                        out_d[0:1, k0 * s:k0 * s + 2 * w2].rearrange(
                            "o (p j) -> (o p) j", p=4), ot[:])

            ilv = int(os.environ.get("ILV", 4))
            g0n = int(os.environ.get("G0N", 0))  # smaller first group
            groups = []
            k0 = 0
            while k0 < nchunk:
                n = min(ilv, nchunk - k0)
                if k0 == 0 and 0 < g0n < n:
                    n = g0n
                groups.append(list(range(k0, k0 + n)))
                k0 += n
            lookahead = int(os.environ.get("LOOKAHEAD", 2))

            phrot = os.environ.get("PHROT", "0") == "1"

            def emit_front(ks, gi=0):
                ph = (gi % 2) if phrot else 0
                xts = [emit_xload(k) for k in ks]
                mts = [(apool.tile([128, 5, cf], fp8, tag="mt", name="mt"),
                        (0 if (ci % 4) < 2 else 1) ^ ph)
                       for ci in range(len(ks))]
                for (mt, v), xt in zip(mts, xts):
                    z0 = emit_conv0(xt)
                    emit_thresh_stage(_veng(0, ENG_S0, v), mt, 0, z0,
                                      thr0p, nthr0)
                return mts

            pending_fc = None
            mts = emit_front(groups[0], 0)
            for gi, ks in enumerate(groups):
                nxt = None
                for li in range(4):
                    zps = []
                    for mt, v in mts:
                        if li == 0:
                            zps.append(emit_conv1(mt, v))
                        else:
                            zps.append(emit_convl(mt, li, v))
                    for (mt, v), zp in zip(mts, zps):
                        emit_thresh_stage(_veng(li + 1, ENG_ST[li], v),
                                          mt, li + 1, zp,
                                          thrl[li], nthrl[li])
                    # interleave the next group's front half-way through so
                    # the s0 wave overlaps this group's tail layers
                    la = (int(os.environ.get("LA0", lookahead))
                          if gi == 0 else lookahead)
                    if li == la and gi + 1 < len(groups):
                        nxt = emit_front(groups[gi + 1], gi + 1)
                if pending_fc is not None:
                    emit_fc(*pending_fc)
                pending_fc = (ks[0], mts, gi % 2)
                mts = nxt
            if pending_fc is not None:
                k0, mts, gp = pending_fc
                for i in range(0, len(mts), 2):
                    emit_fc_batch(k0 + i, mts[i:i + 2], gp)

    nc.finalize()
    return nc


_NC_CACHE = {}
LAST_EXEC_NS = None
_PATCHED = False


def _split_multiwait_json(bir_bytes):
    """Walrus in this toolchain only supports ONE sync-wait per instruction.
    Split any instruction carrying N>1 waits into N-1 preceding single-wait
    NoOps on the same engine (waits are monotone sem-ge checks, so order is
    irrelevant and the split is semantics-preserving)."""
    import json as _json
    d = _json.loads(bir_bytes)
    nsplit = 0
    for fn in d.get("functions", []):
        for blk in fn.get("blocks", []):
            out = []
            for inst in blk.get("instructions", []):
                si = inst.get("sync_info")
                waits = (si or {}).get("on_wait") or []
                if len(waits) > 1:
                    for wi, w in enumerate(waits[:-1]):
                        out.append({
                            "name": f"{inst['name']}-ws{wi}",
                            "opcode": "NoOp",
                            "engine": inst["engine"],
                            "ins": [],
                            "outs": [],
                            "debug": inst.get("debug", 0),
                            "sync_info": {"on_update": [], "on_wait": [w]},
                        })
                        nsplit += 1
                    si["on_wait"] = [waits[-1]]
                out.append(inst)
            blk["instructions"] = out
    if nsplit:
        print(f"[kernel] split {nsplit} extra sync-waits into NoOps",
              file=sys.stderr)
    return _json.dumps(d).encode()


def _install_patches():
    global _PATCHED
    if _PATCHED:
        return
    from concourse import bass_utils, bass2jax
    orig = bass_utils.compile_bir_kernel

    def patched(bir_json, tmpdir, neff_name="file.neff", **kw):
        if isinstance(bir_json, str):
            bir_json = bir_json.encode()
        return orig(_split_multiwait_json(bir_json), tmpdir, neff_name, **kw)

    bass_utils.compile_bir_kernel = patched
    bass2jax.compile_bir_kernel = patched
    _PATCHED = True


def kernel(**inputs):
    _install_patches()
    from concourse.bass_utils import run_bass_kernel_spmd

    x = np.asarray(inputs["x"], np.float32)
    b_total = x.shape[0]
    b_core = b_total // N_CORES
    host = prepare_host_tensors({k: np.asarray(v) for k, v in inputs.items()})

    key = (b_core, S)
    if key not in _NC_CACHE:
        _NC_CACHE[key] = build_nc(b_core, S)
    nc = _NC_CACHE[key]

    x2 = split_fp16(x)
    in_maps = []
    for ci in range(N_CORES):
        m = {"x2": np.ascontiguousarray(x2[ci * b_core:(ci + 1) * b_core])}
        m.update(host)
        in_maps.append(m)

    trace = os.environ.get("KTRACE", "0") == "1"
    try:
        res = run_bass_kernel_spmd(nc, in_maps, core_ids=list(range(N_CORES)),
                                   trace=trace)
    except ModuleNotFoundError:
        # NTFF profile hook unavailable in this container
        res = run_bass_kernel_spmd(nc, in_maps, core_ids=list(range(N_CORES)))
    global LAST_EXEC_NS
    LAST_EXEC_NS = res.exec_time_ns
    outs = [res.results[i]["out"].reshape(-1) for i in range(N_CORES)]
    return np.concatenate(outs).reshape(b_total, 1).astype(np.float32)


if __name__ == "__main__":
    rng = np.random.default_rng(0)
    demo = {
        "x": rng.standard_normal((B_TOTAL, 4, 16), dtype=np.float32),
        "conv0_w": rng.standard_normal((32, 4, 1), dtype=np.float32),
        "conv0_b": rng.standard_normal(32, dtype=np.float32),
        "bn0_g": rng.uniform(0.5, 1.5, 32).astype(np.float32),
        "bn0_b": rng.standard_normal(32, dtype=np.float32),
        "bn0_m": rng.standard_normal(32, dtype=np.float32),
        "bn0_v": np.ones(32, np.float32),
        "convs_w": rng.standard_normal((4, 32, 32, 3), dtype=np.float32),
        "convs_b": rng.standard_normal((4, 32), dtype=np.float32),
        "bns_g": rng.uniform(0.5, 1.5, (4, 32)).astype(np.float32),
        "bns_b": rng.standard_normal((4, 32), dtype=np.float32),
        "bns_m": rng.standard_normal((4, 32), dtype=np.float32),
        "bns_v": np.ones((4, 32), np.float32),
        "fc1_w": rng.standard_normal((64, 512), dtype=np.float32),
        "fc1_b": rng.standard_normal(64, dtype=np.float32),
        "bn5_g": rng.uniform(0.5, 1.5, 64).astype(np.float32),
        "bn5_b": rng.standard_normal(64, dtype=np.float32),
        "bn5_m": rng.standard_normal(64, dtype=np.float32),
        "bn5_v": np.ones(64, np.float32),
        "fc2_w": rng.standard_normal((64, 64), dtype=np.float32),
        "fc2_b": rng.standard_normal(64, dtype=np.float32),
        "bn6_g": rng.uniform(0.5, 1.5, 64).astype(np.float32),
        "bn6_b": rng.standard_normal(64, dtype=np.float32),
        "bn6_m": rng.standard_normal(64, dtype=np.float32),
        "bn6_v": np.ones(64, np.float32),
        "fc3_w": rng.standard_normal((1, 64), dtype=np.float32),
        "fc3_b": rng.standard_normal(1, dtype=np.float32),
    }
    o = kernel(**demo)
    print(o.shape, o[:4, 0])

